# revision 23
# baseline (speedup 1.0000x reference)
"""Trainium2 Bass kernel for nn_AttentionLayer (DIN-style attention scorer).

Math (per batch b):
  info[t] = [q, k[t], q-k[t], q*k[t]]  (256 feats)
  h0 = relu(info @ W0 + b0); h1 = relu(h0 @ W1 + b1); logit[t] = h1 @ Wf + bf
  att = softmax(mask ? logit : NEG); out = sum_t att[t] * v[t]

Restructuring (v2):
  info @ W0 = k @ (C + diag(q)P) + (q@A + b0)  with A=W0a+W0c, C=W0b-W0c,
  P=W0d.  The q-dependent weight W~_b = [C + diag(q_b)P ; q_b@A + b0] is
  host-precomputed per batch ([65,128] bf16) and used as the matmul
  STATIONARY against moving [k^T; ones] ([65,200] per batch).  One K=65
  matmul per batch computes the full first layer including bias.

  mm2 (h1 @ Wf) runs REVERSED: the h1 tile is the stationary and wf is a
  single moving column, so the output free size is 1 (almost free on PE).
  Logits land in [t-partition, batch-column] PSUM tiles P1/P2 directly.

  Softmax: exp on ACT straight from PSUM ([128t,128b] slabs per group of
  128 batches).  The mask is folded into v on the HOST (masked v rows and
  the appended ones-column are zeroed), so no mask ops on device and the
  softmax max-subtraction is dropped (logits are O(3); exp is safe).

  Weighted sum runs REVERSED too: stationary = per-batch v block
  [t,64d + ones-col] (the ones column accumulates S_b = sum_t mask*e),
  moving = exp column [t,1] -> psum US[0:65, b]: rows 0..63 = unnormalized
  output^T, row 64 = softmax denominator.  Final: transpose U and 1/S back
  to [batch-partition, d] via PE transposes and scale on DVE.

Sharding: batch 4096 -> 8 cores x 512. SPMD, no collectives.
"""

import numpy as np
import ml_dtypes

B_TOT, T, D = 4096, 200, 64
H0, H1 = 128, 64
NCORES = 8
BC = B_TOT // NCORES          # 512 batches per core
N = BC * T                    # 102400 (b,t) rows per core
PAIRS = BC // 2               # 256 (2 batches per ps0 tile)
GRP = 128                     # batches per epilogue group
NGRP = BC // GRP              # 4
CHUNK_B = 16                  # batches per kt/wt DMA chunk
NCHUNK = BC // CHUNK_B        # 32

bf16 = ml_dtypes.bfloat16

USE_POOL = True               # 3-way relu split (ACT/DVE/Pool)

import os
KNOB = {
    "ps0": int(os.environ.get("K_PS0", "3")),
    "ps1": int(os.environ.get("K_PS1", "2")),
    "mm1lag": int(os.environ.get("K_MM1LAG", "4")),
    "mm2lag": int(os.environ.get("K_MM2LAG", "4")),
    "relu0eng": os.environ.get("K_R0ENG", "act,dve").split(","),
    "relu1eng": os.environ.get("K_R1ENG", "act,dve,pool").split(","),
    "r1split": int(os.environ.get("K_R1SPLIT", "0")),
    "r0split": int(os.environ.get("K_R0SPLIT", "0")),
    "h0bufs": int(os.environ.get("K_H0BUFS", "7")),
    "h1bufs": int(os.environ.get("K_H1BUFS", "6")),
}

_BUILT = {}


def _build_program():
    import concourse.bacc as bacc
    import concourse.tile as tile
    from concourse import mybir

    fp32 = mybir.dt.float32
    bfl = mybir.dt.bfloat16
    AF = mybir.ActivationFunctionType
    ALU = mybir.AluOpType

    nc = bacc.Bacc("TRN2", target_bir_lowering=False, debug=False,
                   num_devices=NCORES)

    ktD = nc.dram_tensor("kt", [65, N], bfl, kind="ExternalInput").ap()
    wtD = nc.dram_tensor("wt", [65, BC * 128], bfl, kind="ExternalInput").ap()
    vt1D = nc.dram_tensor("vt1", [128, BC * 65], bfl,
                          kind="ExternalInput").ap()
    vt2D = nc.dram_tensor("vt2", [72, BC * 65], bfl,
                          kind="ExternalInput").ap()
    w1D = nc.dram_tensor("w1", [128, 64], bfl, kind="ExternalInput").ap()
    wf2D = nc.dram_tensor("wf2", [128, 1], bfl, kind="ExternalInput").ap()
    b1rD = nc.dram_tensor("b1r", [128, 1], fp32, kind="ExternalInput").ap()
    id64D = nc.dram_tensor("id64", [64, 64], bfl, kind="ExternalInput").ap()
    one11D = nc.dram_tensor("one11", [1, 1], bfl, kind="ExternalInput").ap()
    oD = nc.dram_tensor("o", [BC, D], fp32, kind="ExternalOutput").ap()

    with tile.TileContext(nc) as tc:
        with (
            tc.tile_pool(name="wts", bufs=1) as wpool,
            tc.tile_pool(name="ktp", bufs=4) as ktpool,
            tc.tile_pool(name="wtp", bufs=4) as wtpool,
            tc.tile_pool(name="h0p", bufs=KNOB["h0bufs"]) as h0pool,
            tc.tile_pool(name="h1p", bufs=KNOB["h1bufs"]) as h1pool,
            tc.tile_pool(name="ep", bufs=2) as epool,
            tc.tile_pool(name="fin", bufs=1) as fpool,
            tc.tile_pool(name="pp0", bufs=KNOB["ps0"], space="PSUM") as pp0,
            tc.tile_pool(name="pp1", bufs=KNOB["ps1"], space="PSUM") as pp1,
            tc.tile_pool(name="pl1", bufs=1, space="PSUM") as pl1,
            tc.tile_pool(name="pl2", bufs=1,
                         space="PSUM" if not int(
                             os.environ.get("K_SHARE_P2", "0"))
                         else "SBUF") as pl2,
            tc.tile_pool(name="pus", bufs=1, space="PSUM") as pus,
        ):
            w1_sb = wpool.tile([128, 64], bfl, tag="w1")
            nc.sync.dma_start(out=w1_sb[:], in_=w1D)
            wf2_sb = wpool.tile([128, 1], bfl, tag="wf2")
            nc.sync.dma_start(out=wf2_sb[:], in_=wf2D)
            b1r_sb = wpool.tile([128, 1], fp32, tag="b1r")
            nc.sync.dma_start(out=b1r_sb[:], in_=b1rD)
            id64_sb = wpool.tile([64, 64], bfl, tag="id64")
            nc.sync.dma_start(out=id64_sb[:], in_=id64D)
            one11_sb = wpool.tile([1, 1], bfl, tag="one11")
            nc.sync.dma_start(out=one11_sb[:], in_=one11D)

            vt1_sb = wpool.tile([128, BC * 65], bfl, tag="vt1")
            vt2_sb = wpool.tile([72, BC * 65], bfl, tag="vt2")

            # logits [t-part, batch-col]
            P1 = pl1.tile([128, BC], fp32, tag="P1")
            if int(os.environ.get("K_SHARE_P2", "0")):
                P2 = P1  # TIMING EXPERIMENT ONLY: wrong results
            else:
                P2 = pl2.tile([128, BC], fp32, tag="P2")
            # US: rows 0..63 = unnormalized out^T, row 64 = exp-sum
            US = pus.tile([128, BC], fp32, tag="US")

            # --- element-wise engine load balancer ---
            load = {"act": 3500.0, "dve": 1500.0}
            COST_FULL = {"act": 476.0, "dve": 542.0, "pool": 850.0}
            COST_HALF = {"act": 310.0, "dve": 333.0, "pool": 470.0}
            if USE_POOL:
                load["pool"] = 0.0

            def relu(dst, src, bias=None, engines=("act", "dve", "pool"),
                     cost=COST_FULL):
                engines = [e for e in engines if e in load]
                eng = min(engines, key=lambda e: load[e] + cost[e])
                load[eng] += cost[eng]
                if eng == "act":
                    if bias is None:
                        nc.scalar.activation(dst, src, AF.Relu)
                    else:
                        nc.scalar.activation(dst, src, AF.Relu, bias=bias)
                else:
                    v = nc.vector if eng == "dve" else nc.gpsimd
                    if bias is None:
                        v.tensor_scalar_max(dst, src, 0.0)
                    else:
                        v.tensor_scalar(dst, src, bias, 0.0, ALU.add, ALU.max)

            exp_tiles = {}

            def epilogue_exp(g):
                c0 = GRP * g
                e1 = epool.tile([128, GRP], bfl, tag="e1")
                nc.scalar.activation(e1[:], P1[:, c0:c0 + GRP], AF.Exp)
                e2 = epool.tile([128, GRP], bfl, tag="e2")
                nc.scalar.activation(e2[0:72, :], P2[0:72, c0:c0 + GRP],
                                     AF.Exp)
                exp_tiles[g] = (e1, e2)

            def epilogue_wsum(g):
                c0 = GRP * g
                e1, e2 = exp_tiles.pop(g)
                for lb in range(GRP):
                    B = c0 + lb
                    nc.tensor.matmul(US[0:65, B:B + 1],
                                     vt1_sb[:, 65 * B:65 * B + 65],
                                     e1[:, lb:lb + 1],
                                     start=True, stop=False)
                    nc.tensor.matmul(US[0:65, B:B + 1],
                                     vt2_sb[0:72, 65 * B:65 * B + 65],
                                     e2[0:72, lb:lb + 1],
                                     start=False, stop=True)

            def emit_mm2(q, h1t):
                for j in range(4):
                    par, ci = j // 2, j % 2
                    B = 4 * q + j
                    r0 = 64 * par
                    cc = 200 * ci
                    nc.tensor.matmul(
                        P1[:, B:B + 1],
                        h1t[r0:r0 + 64, cc:cc + 128],
                        wf2_sb[r0:r0 + 64, 0:1],
                        start=True, stop=True)
                    nc.tensor.matmul(
                        P2[0:72, B:B + 1],
                        h1t[r0:r0 + 64, cc + 128:cc + 200],
                        wf2_sb[r0:r0 + 64, 0:1],
                        start=True, stop=True)

            # ---------------- main loop (software pipelined) ----------------
            # stage lags (in pairs): mm1 runs 2 pairs after mm0/relu0,
            # relu1 right after mm1-odd, mm2 4 pairs after its relu1.
            MM1_LAG = KNOB["mm1lag"]
            MM2_LAG = KNOB["mm2lag"]
            h0_tiles = {}
            h1_tiles = {}
            ps1 = None

            def stage_mm0(p):
                lp = p % 8
                ft = kt_tiles[p // 8][:, 400 * lp:400 * lp + 400]
                lb0 = 2 * lp
                wt_t = wt_tiles[p // 8]
                wA = wt_t[:, 128 * lb0:128 * lb0 + 128]
                wB = wt_t[:, 128 * (lb0 + 1):128 * (lb0 + 1) + 128]
                ps0 = pp0.tile([128, 400], fp32, tag="ps0")
                nc.tensor.matmul(ps0[:, 0:200], wA, ft[:, 0:200],
                                 start=True, stop=True)
                nc.tensor.matmul(ps0[:, 200:400], wB, ft[:, 200:400],
                                 start=True, stop=True)
                h0t = h0pool.tile([128, 400], bfl, tag="h0")
                if KNOB["r0split"]:
                    relu(h0t[:, 0:200], ps0[:, 0:200],
                         engines=KNOB["relu0eng"], cost=COST_HALF)
                    relu(h0t[:, 200:400], ps0[:, 200:400],
                         engines=KNOB["relu0eng"], cost=COST_HALF)
                else:
                    relu(h0t[:], ps0[:], engines=KNOB["relu0eng"])
                h0_tiles[p] = h0t

            def stage_mm1(p):
                nonlocal ps1
                h0t = h0_tiles.pop(p)
                if p % 2 == 0:
                    ps1 = pp1.tile([128, 400], fp32, tag="ps1")
                    nc.tensor.matmul(ps1[0:64, :], w1_sb[:], h0t[:],
                                     start=True, stop=True,
                                     tile_position=(0, 0))
                else:
                    nc.tensor.matmul(ps1[64:128, :], w1_sb[:], h0t[:],
                                     start=True, stop=True,
                                     tile_position=(0, 64))
                    h1t = h1pool.tile([128, 400], bfl, tag="h1")
                    if KNOB["r1split"]:
                        relu(h1t[:, 0:200], ps1[:, 0:200], bias=b1r_sb[:],
                             engines=KNOB["relu1eng"], cost=COST_HALF)
                        relu(h1t[:, 200:400], ps1[:, 200:400], bias=b1r_sb[:],
                             engines=KNOB["relu1eng"], cost=COST_HALF)
                    else:
                        relu(h1t[:], ps1[:], bias=b1r_sb[:],
                             engines=KNOB["relu1eng"])
                    h1_tiles[p // 2] = h1t

            kt_tiles = {}
            wt_tiles = {}
            for p in range(PAIRS + MM1_LAG + MM2_LAG + 2):
                if p % 8 == 0 and p < PAIRS:
                    c = p // 8
                    kt_t = ktpool.tile([65, 400 * 8], bfl, tag="kt")
                    nc.sync.dma_start(
                        out=kt_t[:],
                        in_=ktD[:, 3200 * c:3200 * (c + 1)])
                    kt_tiles[c] = kt_t
                    wt_t = wtpool.tile([65, 128 * CHUNK_B], bfl, tag="wt")
                    nc.sync.dma_start(
                        out=wt_t[:],
                        in_=wtD[:, 2048 * c:2048 * (c + 1)])
                    wt_tiles[c] = wt_t
                    # vt prefetch: s 0..3 -> vt1 quarters, s 4..5 -> vt2
                    # halves of the group this chunk belongs to.
                    g = c // 8
                    s = c % 8
                    gc0 = 65 * GRP * g
                    if s < 4:
                        q0 = gc0 + s * (65 * 32)
                        nc.sync.dma_start(
                            out=vt1_sb[:, q0:q0 + 65 * 32],
                            in_=vt1D[:, q0:q0 + 65 * 32])
                    elif s < 6:
                        q0 = gc0 + (s - 4) * (65 * 64)
                        nc.sync.dma_start(
                            out=vt2_sb[0:72, q0:q0 + 65 * 64],
                            in_=vt2D[:, q0:q0 + 65 * 64])
                if p % 64 == 24 and p >= 64 and p // 64 - 1 < NGRP:
                    epilogue_exp(p // 64 - 1)
                if p % 64 == 34 and p >= 64 and p // 64 - 1 < NGRP:
                    epilogue_wsum(p // 64 - 1)

                if p < PAIRS:
                    stage_mm0(p)
                pm = p - MM1_LAG
                if 0 <= pm < PAIRS:
                    stage_mm1(pm)
                qm = (p - MM1_LAG - MM2_LAG) // 2
                if (p - MM1_LAG - MM2_LAG) % 2 == 1 and 0 <= qm < PAIRS // 2:
                    emit_mm2(qm, h1_tiles.pop(qm))

            epilogue_exp(NGRP - 1)
            epilogue_wsum(NGRP - 1)

            # ---------------- final normalize ----------------
            # UT reuses P1's psum bank (same pool slot, P1 is dead here):
            # cols 0:256 = transposed out, 256:260 = transposed sums
            UT = pl1.tile([128, 260], bfl, tag="P1")
            ssb = fpool.tile([1, BC], bfl, tag="ssb")
            nc.scalar.copy(ssb[:], US[64:65, 0:BC])
            ub = fpool.tile([64, BC], bfl, tag="ub")
            nc.scalar.copy(ub[:], US[0:64, 0:BC])
            for g in range(NGRP):
                nc.tensor.transpose(UT[:, 256 + g:257 + g],
                                    ssb[0:1, GRP * g:GRP * (g + 1)],
                                    one11_sb[:])
            recip = fpool.tile([128, NGRP], fp32, tag="recip")
            nc.vector.reciprocal(recip[:], UT[:, 256:260])
            osb = fpool.tile([128, 4 * D], fp32, tag="osb")
            for g in range(NGRP):
                nc.tensor.transpose(UT[:, 64 * g:64 * g + 64],
                                    ub[0:64, GRP * g:GRP * (g + 1)],
                                    id64_sb[:])
                nc.vector.tensor_scalar_mul(osb[:, 64 * g:64 * g + 64],
                                            UT[:, 64 * g:64 * g + 64],
                                            recip[:, g:g + 1])
            nc.sync.dma_start(
                out=oD.rearrange("(g p) d -> p g d", p=128),
                in_=osb[:].rearrange("p (g d) -> p g d", d=D))

    nc.compile()
    return nc


def _get_program():
    if "nc" not in _BUILT:
        _BUILT["nc"] = _build_program()
    return _BUILT["nc"]


def _prep_core(c, q, k, v, mask, W0, b0, W1, b1, Wf):
    s = slice(c * BC, (c + 1) * BC)
    qc = q[s]                      # [BC, 64] f32
    kc = k[s]                      # [BC, T, 64]
    vc = v[s]
    mc = mask[s]                   # [BC, T] int32

    kt = np.empty((65, N), dtype=bf16)
    kt[0:64] = kc.reshape(N, D).T.astype(bf16)
    kt[64] = bf16(1.0)

    A = W0[0:64] + W0[128:192]
    C = W0[64:128] - W0[128:192]
    P = W0[192:256]
    wt = np.empty((65, BC, 128), dtype=np.float32)
    wt[0:64] = C[:, None, :] + qc.T[:, :, None] * P[:, None, :]
    wt[64] = qc @ A + b0

    mf = mc.astype(np.float32)[:, :, None]          # [BC, T, 1]
    ve = np.concatenate([vc * mf, mf], axis=2)      # [BC, T, 65]
    vt = np.ascontiguousarray(
        ve.transpose(1, 0, 2).reshape(T, BC * 65)).astype(bf16)

    return {
        "kt": kt,
        "wt": wt.reshape(65, BC * 128).astype(bf16),
        "vt1": np.ascontiguousarray(vt[0:128]),
        "vt2": np.ascontiguousarray(vt[128:200]),
        "w1": W1.astype(bf16),
        "wf2": np.vstack([Wf, Wf]).astype(bf16),
        "b1r": np.tile(b1.astype(np.float32), 2).reshape(128, 1),
        "id64": np.eye(64, dtype=np.float32).astype(bf16),
        "one11": np.ones((1, 1), dtype=bf16),
    }


def run(q, k, v, mask, W0, b0, W1, b1, Wf, bf, trace=False):
    from concourse.bass_utils import run_bass_kernel_spmd

    nc = _get_program()
    q = np.asarray(q, dtype=np.float32)
    k = np.asarray(k, dtype=np.float32)
    v = np.asarray(v, dtype=np.float32)
    mask = np.asarray(mask)
    in_maps = [
        _prep_core(c, q, k, v, mask,
                   np.asarray(W0, np.float32), np.asarray(b0, np.float32),
                   np.asarray(W1, np.float32), np.asarray(b1, np.float32),
                   np.asarray(Wf, np.float32))
        for c in range(NCORES)
    ]
    res = run_bass_kernel_spmd(nc, in_maps, list(range(NCORES)), trace=trace)
    out = np.concatenate([res.results[c]["o"] for c in range(NCORES)], axis=0)
    return np.ascontiguousarray(out.astype(np.float32)), res


def kernel(q, k, v, mask, W0, b0, W1, b1, Wf, bf):
    out, _ = run(q, k, v, mask, W0, b0, W1, b1, Wf, bf, trace=False)
    return out
